# revision 4
# baseline (speedup 1.0000x reference)
"""Trainium2 Bass kernel for nn_CoefficientDecoder.

reference computation (all f32):
    h = relu(x @ W1.T + b1)         x:[B,256] -> h:[B,64]
    h = h @ Wd3.T + bd3; h @ Wd2.T + bd2; h @ Wd1.T + bd1   (all linear)
    z = h @ W2.T + b2               [B,512]
    out = z @ bases                 bases:[512,4096] -> out:[B,4096]

Key restructuring (the "headroom" of this problem): everything after the
ReLU is linear, so associativity collapses it:

    out = h @ (W2eff.T @ bases) + b2eff @ bases
        = [h | 1] @ ([W2eff | b2eff].T @ bases)      (augmented form)
        = h_aug[B,65] @ Beff_aug[65,4096]

with W2eff = W2@Wd1@Wd2@Wd3 [512,64], b2eff the folded bias (host-side,
float64).  Beff_aug = W2aug.T @ bases is computed ON DEVICE once per core
(16.4k PE cycles) from f16 bases; the per-batch GEMM then has K=65
instead of K=512 -> ~8x fewer FLOPs than the direct z@bases form.

Sharding: pure data-parallel over batch across 8 cores (1024 rows/core).

Everything on the wire is f16 (rel-err budget is 2e-2; measured ~6e-4):
  per-core DMA = bases 4MB + xT 0.5MB + out 8MB ~= 12.5MB @ 360GB/s ~ 35us
  per-core PE  ~= 60k cycles ~= 25us @ 2.4GHz
so the kernel is DMA-bound; the schedule streams: bases chunk s ->
Beff[:,s] -> out[:, s] column (8 batch tiles) -> stores, so output DMA
overlaps input DMA from ~2us in.

Queue split: loads consts/xT on ACT queue, bases on SP queue; out stores
alternate SP/gpsimd (HWDGE is SP+ACT only; gpsimd = SWDGE); PSUM->SBUF
copies alternate DVE/ACT.

`repeat` wraps the body in a hardware For_i loop - used only for timing
(amortizes the ~100 ms axon dispatch overhead).
"""

import numpy as np

import concourse.bass as bass
import concourse.tile as tile
from concourse import bacc, mybir
from concourse.bass import ts
from concourse.bass_utils import run_bass_kernel_spmd

N_CORES = 8
B, IN_F, HID, NB, SEQ = 8192, 256, 64, 512, 4096
B_LOC = B // N_CORES            # 1024 batch rows per core
HA = HID + 1                    # augmented hidden (h | 1)

F32 = mybir.dt.float32
F16 = mybir.dt.float16

# packed f16 constant layout [128, NCONST]:
#   cols [0, 128):   W1.T as 2 k-chunks of 64   (partition = in_feature % 128)
#   cols [128, 388): W2aug [512,65] as 4 k-chunks of 65 (partition = NB % 128)
C_W1 = 0
C_W2A = 128
NCONST = C_W2A + 4 * HA

KC = IN_F // 128        # 2 k-chunks for layer 1
ZC = NB // 128          # 4 k-chunks for the Beff matmul
NJ = B_LOC // 512       # 2 batch chunks for the MLP moving dim
MM = B_LOC // 128       # 8 batch tiles for the final GEMM
SC = SEQ // 512         # 8 seq chunks

_CACHE = {}


def _build(repeat: int = 1):
    nc = bacc.Bacc(
        "TRN2",
        target_bir_lowering=False,
        debug=False,
        enable_asserts=False,
        num_devices=N_CORES,
    )

    xT_d = nc.declare_dram_parameter("xT", [IN_F, B_LOC], F16, isOutput=False)
    consts_d = nc.declare_dram_parameter("consts", [128, NCONST], F16, isOutput=False)
    b1_d = nc.declare_dram_parameter("b1f", [HID, 1], F32, isOutput=False)
    bases_d = nc.declare_dram_parameter("bases", [NB, SEQ], F16, isOutput=False)
    out_d = nc.declare_dram_parameter("out", [B_LOC, SEQ], F16, isOutput=True)

    relu = mybir.ActivationFunctionType.Relu
    copyf = mybir.ActivationFunctionType.Copy

    with tile.TileContext(nc) as tc:
        with (
            tc.tile_pool(name="const", bufs=1) as constp,
            tc.tile_pool(name="bases", bufs=1) as basesp,
            tc.tile_pool(name="hb", bufs=1) as hbp,
            tc.tile_pool(name="outsb", bufs=6) as outsbp,
            tc.tile_pool(name="mlp_ps", bufs=2, space="PSUM") as mlpp,
            tc.tile_pool(name="beff_ps", bufs=2, space="PSUM") as beffpp,
            tc.tile_pool(name="out_ps", bufs=4, space="PSUM") as outpp,
        ):
            def body():
                # ---- input DMAs: small consts/x on the ACT queue, bases
                # streaming on the SP queue ----
                craw = constp.tile([128, NCONST], F16, tag="craw")
                b1 = constp.tile([HID, 1], F32, tag="b1")
                xT_sb = constp.tile([128, KC, B_LOC], F16, tag="xT")
                nc.scalar.dma_start(craw[:], consts_d[:])
                nc.scalar.dma_start(b1[:], b1_d[:])
                xT_pkn = xT_d.rearrange("(k p) n -> p k n", p=128)
                nc.scalar.dma_start(xT_sb[:], xT_pkn[:])

                bases_pcn = bases_d.rearrange("(c p) n -> p c n", p=128)
                bases_sb = []
                for s in range(SC):
                    t = basesp.tile([128, ZC, 512], F16, tag=f"bases{s}")
                    nc.sync.dma_start(t[:], bases_pcn[:, :, ts(s, 512)])
                    bases_sb.append(t)

                w1t = (craw[:, C_W1 : C_W1 + HID], craw[:, C_W1 + HID : C_W1 + 2 * HID])
                w2a = [craw[:, C_W2A + k * HA : C_W2A + (k + 1) * HA] for k in range(ZC)]

                # ---- h_aug.T [65, B_LOC]: row 64 = ones, rows 0..63 = relu MLP ----
                hT = hbp.tile([HA, B_LOC], F16, tag="hT")
                nc.vector.memset(hT[HID : HID + 1, :], 1.0)
                for j in range(NJ):
                    hp = mlpp.tile([HID, 512], F32, tag="mlp")
                    for k in range(KC):
                        nc.tensor.matmul(
                            hp[:],
                            w1t[k],
                            xT_sb[:, k, ts(j, 512)],
                            start=(k == 0),
                            stop=(k == KC - 1),
                        )
                    nc.scalar.activation(hT[:HID, ts(j, 512)], hp[:], relu, bias=b1)

                # ---- per seq chunk: Beff_aug column block, then the 8 batch
                # tiles of the output column (streams behind the bases DMA) ----
                beff = hbp.tile([HA, SEQ], F16, tag="beff")
                for s in range(SC):
                    bp = beffpp.tile([HA, 512], F32, tag="beffp")
                    for k in range(ZC):
                        nc.tensor.matmul(
                            bp[:],
                            w2a[k],
                            bases_sb[s][:, k, :],
                            start=(k == 0),
                            stop=(k == ZC - 1),
                        )
                    nc.scalar.activation(beff[:, ts(s, 512)], bp[:], copyf)

                    for m in range(MM):
                        op = outpp.tile([128, 512], F32, tag="op")
                        nc.tensor.matmul(
                            op[:], hT[:, ts(m, 128)], beff[:, ts(s, 512)],
                            start=True, stop=True,
                        )
                        ob = outsbp.tile([128, 512], F16, tag="ob")
                        if m % 2 == 0:
                            nc.vector.tensor_copy(ob[:], op[:])
                        else:
                            nc.scalar.activation(ob[:], op[:], copyf)
                        dma_eng = (nc.sync, nc.gpsimd)[m % 2]
                        dma_eng.dma_start(out_d[ts(m, 128), ts(s, 512)], ob[:])

            if repeat == 1:
                body()
            else:
                with tc.For_i(0, repeat, 1):
                    body()

    nc.compile()
    return nc


def _get_nc(repeat: int = 1):
    if repeat not in _CACHE:
        _CACHE[repeat] = _build(repeat)
    return _CACHE[repeat]


def _pack_consts(W1, b1, Wd1, bd1, Wd2, bd2, Wd3, bd3, W2, b2):
    # fold the linear chain in float64:
    #   z = h@(W2@Wd1@Wd2@Wd3).T + b2eff;  out = z@bases = h_aug @ (W2aug.T@bases)
    W1 = W1.astype(np.float64); W2 = W2.astype(np.float64)
    Wd1 = Wd1.astype(np.float64); Wd2 = Wd2.astype(np.float64); Wd3 = Wd3.astype(np.float64)
    W2eff = W2 @ Wd1 @ Wd2 @ Wd3                      # [512, 64]
    b2eff = b2 + (bd3 @ Wd2.T @ Wd1.T + bd2 @ Wd1.T + bd1) @ W2.T
    W2aug = np.concatenate([W2eff, b2eff[:, None]], axis=1)  # [512, 65]
    c = np.zeros((128, NCONST), np.float16)
    W1T = W1.T  # [256, 64]
    for k in range(KC):
        c[:, C_W1 + k * HID : C_W1 + (k + 1) * HID] = W1T[k * 128 : (k + 1) * 128]
    for k in range(ZC):
        c[:, C_W2A + k * HA : C_W2A + (k + 1) * HA] = W2aug[k * 128 : (k + 1) * 128]
    return c, np.asarray(b1, np.float32).reshape(HID, 1)


def _in_maps(x, W1, b1, Wd1, bd1, Wd2, bd2, Wd3, bd3, W2, b2, bases):
    c, b1f = _pack_consts(W1, b1, Wd1, bd1, Wd2, bd2, Wd3, bd3, W2, b2)
    common = {
        "consts": c,
        "b1f": b1f,
        "bases": np.ascontiguousarray(bases.astype(np.float16)),
    }
    maps = []
    for i in range(N_CORES):
        m = dict(common)
        m["xT"] = np.ascontiguousarray(x[i * B_LOC : (i + 1) * B_LOC].T.astype(np.float16))
        maps.append(m)
    return maps


def run(inputs: dict, repeat: int = 1, **run_kwargs):
    """Shard, execute on 8 cores, gather. Returns (out, BassKernelResults)."""
    nc = _get_nc(repeat)
    in_maps = _in_maps(**{k: np.asarray(v) for k, v in inputs.items()})
    res = run_bass_kernel_spmd(nc, in_maps, list(range(N_CORES)), **run_kwargs)
    shards = [np.asarray(res.results[i]["out"], dtype=np.float32) for i in range(N_CORES)]
    out = np.concatenate(shards, axis=0)
    return out, res


def kernel(**inputs) -> np.ndarray:
    out, _ = run(inputs)
    return out
